# revision 1
# baseline (speedup 1.0000x reference)
"""Trainium2 Bass kernel for ContrastiveAffinityLossWithMemory.

Strategy (B=4096, D=512, C=4096, dd=384, 8 cores):
  - Host: closed-form of the sequential scatter-EMA memory update (it only
    feeds the loss through the normalized bank m and weights w), gather of
    lookup rows by label pre-scaled to u = w_c*(1-t), and the analytic
    pieces sum(w*d^2) = 2W - 2 x.s_m and sum(d^2) over pairs.
  - Device (SPMD, data-parallel over batch rows, 512 rows/core): the two
    O(B^2 d)/O(B C d) matmuls in bf16 on PE, d = sqrt(2-2cos) on ScalarE
    with fused free-axis accumulation, and sum_c u*d via one fused
    tensor_tensor_reduce on VectorE. Operands are pre-scaled by 0.996 so
    2-2cos stays strictly positive (sqrt-safe); the induced error on the
    final scalar is ~1e-6 relative.
  - Host: combine per-core partial sums (f64) into the final scalar.
"""
import numpy as np
import ml_dtypes

ALPHA = 0.7
DECAY = 0.01
CUR_TIME = 1.0
EPS = 1e-12
MARGIN = 4.0
B, D, C = 4096, 512, 4096
DD = 384
N_CORES = 8
RPC = B // N_CORES          # rows per core = 512
RB = RPC // 128             # row blocks per core = 4
CT_B = B // 512             # col tiles, batch side = 8
CT_M = C // 512             # col tiles, mem side = 8
KC = DD // 128              # contraction chunks = 3
SCALE = 0.996               # operand pre-scale; keeps device cos < 1

TRACE = False               # test harness may flip these
LAST_RESULTS = {}

_NC_CACHE = {}


# ---------------------------------------------------------------- host math
def _l2norm(a):
    n = np.maximum(np.linalg.norm(a, axis=-1, keepdims=True), EPS)
    return (a / n).astype(np.float32)


def _bank_update(l, yp, mem_embeddings, mem_timestamps, mem_initialized):
    """Closed form of the per-sample conditional scatter-EMA over valid
    samples (l already filtered/clipped to [0, C))."""
    Cc, dd = mem_embeddings.shape
    n = l.shape[0]
    init0 = mem_initialized.astype(bool)

    counts = np.bincount(l, minlength=Cc)
    if n:
        order = np.argsort(l, kind="stable")
        ls = l[order]
        grp_start = np.r_[0, np.flatnonzero(np.diff(ls)) + 1]
        start_of_grp = np.repeat(grp_start, np.diff(np.r_[grp_start, n]))
        rank_sorted = np.arange(n) - start_of_grp
        k_i = counts[ls]
        pw = (1.0 - ALPHA) ** (k_i - 1 - rank_sorted).astype(np.float64)
        coef = ALPHA * pw
        first_uninit = (rank_sorted == 0) & (~init0[ls])
        coef[first_uninit] = pw[first_uninit]
        contrib = coef[:, None].astype(np.float32) * yp[order]
        seg = np.add.reduceat(contrib, grp_start, axis=0)
        acc = np.zeros((Cc, dd), dtype=np.float32)
        acc[ls[grp_start]] = seg
    else:
        acc = np.zeros((Cc, dd), dtype=np.float32)

    hit = counts > 0
    coef_old = np.where(hit, np.where(init0, (1.0 - ALPHA) ** counts, 0.0),
                        1.0).astype(np.float32)
    emb_new = coef_old[:, None] * mem_embeddings + acc
    init_new = init0 | hit
    ts_new = np.where(hit, np.float32(CUR_TIME),
                      mem_timestamps).astype(np.float32)
    return emb_new, init_new, ts_new


def _numpy_fallback(y_true, y_pred, lookup, mem_embeddings, mem_timestamps,
                    mem_initialized):
    """Faithful numpy port of the reference; used only if the inputs violate
    the fast path's assumptions (e.g. -1/background labels)."""
    b = y_pred.shape[0]
    c = lookup.shape[0]
    dd = int(y_pred.shape[1] * 0.75)
    yp = y_pred[:, :dd].astype(np.float32)
    l = np.asarray(y_true).astype(np.int64)
    valid = (l >= 0) & (l < c)
    lc = np.clip(l, 0, c - 1)

    emb, init, ts = _bank_update(lc[valid], yp[valid], mem_embeddings,
                                 mem_timestamps, mem_initialized)
    x = _l2norm(yp)
    cos = x @ x.T
    sqd = np.clip(2.0 - 2.0 * cos, 0.0, None)
    tri = np.triu(np.ones((b, b), bool), k=1)
    dist = np.sqrt(np.where(tri, sqd, 1.0))
    is_bg = l == -1
    both = is_bg[:, None] & is_bg[None, :]
    one = is_bg[:, None] ^ is_bg[None, :]
    tsim = np.where(both, 0.2, np.where(one, 0.01, 0.0))
    md = np.maximum(MARGIN - dist, 0.0)
    pair = tsim * dist**2 + (1.0 - tsim) * md**2
    n_pairs = b * (b - 1) // 2
    batch_loss = np.where(tri, pair, 0.0).sum(dtype=np.float64) / n_pairs

    m = np.where(init[:, None], _l2norm(emb), 0.0).astype(np.float32)
    cos_m = x @ m.T
    sqd_m = np.clip(2.0 - 2.0 * cos_m, 0.0, None)
    dist_m = np.sqrt(np.maximum(sqd_m, EPS))
    tsim_m = lookup[lc]
    w = (np.exp(-DECAY * (CUR_TIME - ts)) * init).astype(np.float32)
    md_m = np.maximum(MARGIN - dist_m, 0.0)
    term = (tsim_m * dist_m**2 + (1.0 - tsim_m) * md_m**2) * w[None, :]
    n_init = max(int(init.sum()), 1)
    per_sample = np.where(init[None, :], term, 0.0).sum(
        axis=1, dtype=np.float64) / n_init
    n_valid = max(int(valid.sum()), 1)
    mem_loss = (per_sample * valid).sum(dtype=np.float64) / n_valid
    return np.float32(0.7 * batch_loss + 0.3 * mem_loss)


def _host_prep(y_true, y_pred, lookup, mem_embeddings, mem_timestamps,
               mem_initialized):
    bf16 = ml_dtypes.bfloat16
    l = np.asarray(y_true).astype(np.int64)
    yp = np.ascontiguousarray(y_pred[:, :DD]).astype(np.float32)

    emb, init, ts = _bank_update(l, yp, mem_embeddings, mem_timestamps,
                                 mem_initialized)
    m = np.where(init[:, None], _l2norm(emb), 0.0).astype(np.float32)
    w = (np.exp(-DECAY * (CUR_TIME - ts)) * init).astype(np.float32)
    n_init = max(int(init.sum()), 1)

    x = _l2norm(yp)
    xs = (x * SCALE).astype(bf16)             # [B, DD]
    ms = (m * SCALE).astype(bf16)             # [C, DD]

    t = lookup[l]                             # [B, C] f32 host gather
    w64 = w.astype(np.float64)
    u32 = w[None, :] * (1.0 - t)                                   # [B, C]
    R = u32.sum(axis=1, dtype=np.float64)                          # [B]
    u16 = u32.astype(np.float16)

    xt3 = np.ascontiguousarray(xs.T).reshape(KC, 128, B)
    mt3 = np.ascontiguousarray(ms.T).reshape(KC, 128, C)

    in_maps = []
    for k in range(N_CORES):
        rows = slice(k * RPC, (k + 1) * RPC)
        in_maps.append({
            "xt3": xt3,
            "mt3": mt3,
            "xtk3": np.ascontiguousarray(xs[rows].T).reshape(KC, 128, RPC),
            "u": np.ascontiguousarray(u16[rows]),
        })

    # analytic pieces (f64)
    xs64 = xs.astype(np.float64)
    cos_ii = (xs64 * xs64).sum(axis=1)
    Sd_diag = np.sqrt(np.maximum(2.0 - 2.0 * cos_ii, 0.0)).sum()
    s_vec = xs64.sum(axis=0)
    T2_upper = (B * (B - 1) // 2) * 2.0 - (s_vec @ s_vec - cos_ii.sum())

    W = w64.sum()
    s_m = (w64[:, None] * m.astype(np.float64)).sum(axis=0)
    xdots = x.astype(np.float64) @ s_m

    meta = dict(Sd_diag=Sd_diag, T2_upper=T2_upper, W=W, xdots=xdots, R=R,
                n_init=n_init, n_valid=B)
    return in_maps, meta


def _assemble(results, meta):
    S_all_d = 0.0
    q = np.zeros(B, dtype=np.float64)
    for k, res in enumerate(results):
        s_acc = np.asarray(res["s_acc"], dtype=np.float64)   # [128, 32]
        q_acc = np.asarray(res["q_acc"], dtype=np.float64)   # [128, 32]
        S_all_d += s_acc.sum()
        qk = q_acc.reshape(128, RB, CT_M).sum(axis=2)        # [128, RB]
        for rb in range(RB):
            rows = slice(k * RPC + rb * 128, k * RPC + (rb + 1) * 128)
            q[rows] = qk[:, rb]

    n_pairs = B * (B - 1) // 2
    Sd_upper = (S_all_d - meta["Sd_diag"]) / 2.0
    batch_sum = 16.0 * n_pairs - 8.0 * Sd_upper + meta["T2_upper"]
    batch_loss = batch_sum / n_pairs

    per_i = (2.0 * meta["W"] - 2.0 * meta["xdots"]) + 16.0 * meta["R"] - 8.0 * q
    mem_loss = per_i.sum() / meta["n_init"] / meta["n_valid"]
    return np.float32(0.7 * batch_loss + 0.3 * mem_loss)


# ---------------------------------------------------------------- device
def _build_nc():
    if "nc" in _NC_CACHE:
        return _NC_CACHE["nc"]
    import concourse.bacc as bacc
    import concourse.bass as bass
    import concourse.mybir as mybir
    import concourse.tile as tile
    from concourse._compat import get_trn_type

    f32 = mybir.dt.float32
    bf16 = mybir.dt.bfloat16
    f16 = mybir.dt.float16

    nc = bacc.Bacc(get_trn_type() or "TRN2", target_bir_lowering=False,
                   debug=False)

    xt3 = nc.dram_tensor("xt3", [KC, 128, B], bf16, kind="ExternalInput")
    mt3 = nc.dram_tensor("mt3", [KC, 128, C], bf16, kind="ExternalInput")
    xtk3 = nc.dram_tensor("xtk3", [KC, 128, RPC], bf16, kind="ExternalInput")
    u = nc.dram_tensor("u", [RPC, C], f16, kind="ExternalInput")
    s_out = nc.dram_tensor("s_acc", [128, RB * CT_B], f32,
                           kind="ExternalOutput")
    q_out = nc.dram_tensor("q_acc", [128, RB * CT_M], f32,
                           kind="ExternalOutput")

    with tile.TileContext(nc) as tc:
        with (
            tc.tile_pool(name="const", bufs=1) as const,
            tc.tile_pool(name="psum", bufs=6, space="PSUM") as psum,
            tc.tile_pool(name="work", bufs=4) as work,
        ):
            xall = []
            mall = []
            xk = []
            for kc in range(KC):
                ta = const.tile([128, B], bf16, tag=f"xall{kc}")
                nc.sync.dma_start(ta[:], xt3[kc])
                xall.append(ta)
                tm = const.tile([128, C], bf16, tag=f"mall{kc}")
                nc.sync.dma_start(tm[:], mt3[kc])
                mall.append(tm)
                tk = const.tile([128, RPC], bf16, tag=f"xk{kc}")
                nc.sync.dma_start(tk[:], xtk3[kc])
                xk.append(tk)

            s_acc = const.tile([128, RB * CT_B], f32, tag="s_acc")
            q_acc = const.tile([128, RB * CT_M], f32, tag="q_acc")
            bias2 = const.tile([128, 1], f32, tag="bias2")
            nc.vector.memset(bias2[:], 2.0)

            for rb in range(RB):
                rsl = slice(rb * 128, (rb + 1) * 128)
                for ct in range(CT_B):
                    csl = slice(ct * 512, (ct + 1) * 512)
                    ps = psum.tile([128, 512], f32, tag="ps")
                    for kc in range(KC):
                        nc.tensor.matmul(ps[:], xk[kc][:, rsl],
                                         xall[kc][:, csl],
                                         start=(kc == 0), stop=(kc == KC - 1))
                    col = rb * CT_B + ct
                    db = work.tile([128, 512], bf16, tag="db")
                    nc.scalar.activation(
                        db[:], ps[:], mybir.ActivationFunctionType.Sqrt,
                        bias=bias2[:], scale=-2.0,
                        accum_out=s_acc[:, col:col + 1])
                for ct in range(CT_M):
                    csl = slice(ct * 512, (ct + 1) * 512)
                    ps = psum.tile([128, 512], f32, tag="ps")
                    for kc in range(KC):
                        nc.tensor.matmul(ps[:], xk[kc][:, rsl],
                                         mall[kc][:, csl],
                                         start=(kc == 0), stop=(kc == KC - 1))
                    dm = work.tile([128, 512], f32, tag="dm")
                    nc.scalar.activation(
                        dm[:], ps[:], mybir.ActivationFunctionType.Sqrt,
                        bias=bias2[:], scale=-2.0)
                    ut = work.tile([128, 512], f16, tag="ut")
                    nc.sync.dma_start(ut[:], u[rsl, csl])
                    junk = work.tile([128, 512], f32, tag="junk")
                    col = rb * CT_M + ct
                    nc.vector.tensor_tensor(junk[:], dm[:], ut[:],
                                            mybir.AluOpType.mult)
                    nc.vector.tensor_reduce(q_acc[:, col:col + 1], junk[:],
                                            mybir.AxisListType.XYZW,
                                            mybir.AluOpType.add)

            nc.sync.dma_start(s_out[:], s_acc[:])
            nc.sync.dma_start(q_out[:], q_acc[:])

    nc.compile()
    _NC_CACHE["nc"] = nc
    return nc


def kernel(y_true, y_pred, lookup, mem_embeddings, mem_timestamps,
           mem_initialized):
    y_true = np.asarray(y_true)
    y_pred = np.asarray(y_pred, dtype=np.float32)
    lookup = np.asarray(lookup, dtype=np.float32)
    mem_embeddings = np.asarray(mem_embeddings, dtype=np.float32)
    mem_timestamps = np.asarray(mem_timestamps, dtype=np.float32)
    mem_initialized = np.asarray(mem_initialized, dtype=np.int32)

    l = y_true.astype(np.int64)
    if (y_pred.shape != (B, D) or lookup.shape != (C, C)
            or not ((l >= 0) & (l < C)).all()):
        return _numpy_fallback(y_true, y_pred, lookup, mem_embeddings,
                               mem_timestamps, mem_initialized)

    from concourse.bass_utils import run_bass_kernel_spmd

    nc = _build_nc()
    in_maps, meta = _host_prep(y_true, y_pred, lookup, mem_embeddings,
                               mem_timestamps, mem_initialized)
    res = run_bass_kernel_spmd(nc, in_maps, list(range(N_CORES)),
                               trace=TRACE)
    LAST_RESULTS["bass"] = res
    return _assemble(res.results, meta)



# revision 21
# speedup vs baseline: 60.3353x; 60.3353x over previous
"""Trainium2 Bass kernel for ContrastiveAffinityLossWithMemory.

Strategy (B=4096, D=512, C=4096, dd=384, 8 cores):
  Host: closed-form scatter-EMA bank update; normalized bank m and weights
  w; per-label lookup gather pre-scaled to u = w_c*(1-t); analytic pieces
  (sum of w*d^2 via 2W - 2 x.s_m, the d^2 pair sums, per-sample R_i).

  Device (SPMD, 8 cores): the only O(B^2 d)/O(B C d) work — the two cosine
  matmul families, d = sqrt(2-2cos) on ScalarE, and sum_c u*d via one fused
  tensor_tensor_reduce on VectorE.

  v2 layout (vs the v1 full-matrix kernel):
   - Upper-triangle batch term via per-core column ROTATION: core k receives
     x^T columns rotated by k*512, and always computes col-tiles 0..4
     (its own diagonal 512-block, three "pure" upper blocks, and the
     antipodal distance-4 block which two cores compute redundantly and the
     host halves). 20 [128,512] batch tiles per core instead of 32.
   - Memory term trimmed to initialized classes only: host permutes classes
     so w>0 ones are contiguous, padded to a multiple of 512 (CM columns).
   - Three consolidated input DMAs per core (x-rot, m, u), each contiguous
     multi-KB lines; no separate weights tensor (weights are slices of the
     x-rot SBUF tile).
   - PSUM used as [128,2048] 4-bank big tiles; wide activations amortize
     the ~370ns fixed overhead; DVE does fused (d*u -> sum) with all-16-bit
     operands for the 2x mode.
  Host combines per-core partial sums in f64 into the final scalar.

  _build_nc(reps=N) unrolls the whole per-core program N times (identical,
  idempotent writes) so test.py can measure hardware time differentially:
  (T(reps=R) - T(reps=1)) / (R-1) cancels the multi-ms per-dispatch axon
  overhead that an absolute measurement cannot separate.
"""
import numpy as np
import ml_dtypes

ALPHA = 0.7
DECAY = 0.01
CUR_TIME = 1.0
EPS = 1e-12
MARGIN = 4.0
B, D, C = 4096, 512, 4096
DD = 384
N_CORES = 8
G = N_CORES                 # row groups (512 rows each)
RPC = B // N_CORES          # rows per core = 512
RB = RPC // 128             # row blocks per core = 4
KC = DD // 128              # contraction chunks = 3
NBT = 5                     # batch col tiles per core (rotated 0..4)
BCOLS = NBT * 512           # 2560 rotated x columns shipped per core
SCALE = 0.996               # operand pre-scale; keeps device cos < 1

TRACE = False               # test harness may flip these
LAST_RESULTS = {}

_NC_CACHE = {}


# ---------------------------------------------------------------- host math
def _l2norm(a):
    n = np.maximum(np.linalg.norm(a, axis=-1, keepdims=True), EPS)
    return (a / n).astype(np.float32)


def _bank_update(l, yp, mem_embeddings, mem_timestamps, mem_initialized):
    """Closed form of the per-sample conditional scatter-EMA over valid
    samples (l already filtered/clipped to [0, C))."""
    Cc, dd = mem_embeddings.shape
    n = l.shape[0]
    init0 = mem_initialized.astype(bool)

    counts = np.bincount(l, minlength=Cc)
    if n:
        order = np.argsort(l, kind="stable")
        ls = l[order]
        grp_start = np.r_[0, np.flatnonzero(np.diff(ls)) + 1]
        start_of_grp = np.repeat(grp_start, np.diff(np.r_[grp_start, n]))
        rank_sorted = np.arange(n) - start_of_grp
        k_i = counts[ls]
        pw = (1.0 - ALPHA) ** (k_i - 1 - rank_sorted).astype(np.float64)
        coef = ALPHA * pw
        first_uninit = (rank_sorted == 0) & (~init0[ls])
        coef[first_uninit] = pw[first_uninit]
        contrib = coef[:, None].astype(np.float32) * yp[order]
        seg = np.add.reduceat(contrib, grp_start, axis=0)
        acc = np.zeros((Cc, dd), dtype=np.float32)
        acc[ls[grp_start]] = seg
    else:
        acc = np.zeros((Cc, dd), dtype=np.float32)

    hit = counts > 0
    coef_old = np.where(hit, np.where(init0, (1.0 - ALPHA) ** counts, 0.0),
                        1.0).astype(np.float32)
    emb_new = coef_old[:, None] * mem_embeddings + acc
    init_new = init0 | hit
    ts_new = np.where(hit, np.float32(CUR_TIME),
                      mem_timestamps).astype(np.float32)
    return emb_new, init_new, ts_new


def _numpy_fallback(y_true, y_pred, lookup, mem_embeddings, mem_timestamps,
                    mem_initialized):
    """Faithful numpy port of the reference; used only if the inputs violate
    the fast path's assumptions (e.g. -1/background labels)."""
    b = y_pred.shape[0]
    c = lookup.shape[0]
    dd = int(y_pred.shape[1] * 0.75)
    yp = y_pred[:, :dd].astype(np.float32)
    l = np.asarray(y_true).astype(np.int64)
    valid = (l >= 0) & (l < c)
    lc = np.clip(l, 0, c - 1)

    emb, init, ts = _bank_update(lc[valid], yp[valid], mem_embeddings,
                                 mem_timestamps, mem_initialized)
    x = _l2norm(yp)
    cos = x @ x.T
    sqd = np.clip(2.0 - 2.0 * cos, 0.0, None)
    tri = np.triu(np.ones((b, b), bool), k=1)
    dist = np.sqrt(np.where(tri, sqd, 1.0))
    is_bg = l == -1
    both = is_bg[:, None] & is_bg[None, :]
    one = is_bg[:, None] ^ is_bg[None, :]
    tsim = np.where(both, 0.2, np.where(one, 0.01, 0.0))
    md = np.maximum(MARGIN - dist, 0.0)
    pair = tsim * dist**2 + (1.0 - tsim) * md**2
    n_pairs = b * (b - 1) // 2
    batch_loss = np.where(tri, pair, 0.0).sum(dtype=np.float64) / n_pairs

    m = np.where(init[:, None], _l2norm(emb), 0.0).astype(np.float32)
    cos_m = x @ m.T
    sqd_m = np.clip(2.0 - 2.0 * cos_m, 0.0, None)
    dist_m = np.sqrt(np.maximum(sqd_m, EPS))
    tsim_m = lookup[lc]
    w = (np.exp(-DECAY * (CUR_TIME - ts)) * init).astype(np.float32)
    md_m = np.maximum(MARGIN - dist_m, 0.0)
    term = (tsim_m * dist_m**2 + (1.0 - tsim_m) * md_m**2) * w[None, :]
    n_init = max(int(init.sum()), 1)
    per_sample = np.where(init[None, :], term, 0.0).sum(
        axis=1, dtype=np.float64) / n_init
    n_valid = max(int(valid.sum()), 1)
    mem_loss = (per_sample * valid).sum(dtype=np.float64) / n_valid
    return np.float32(0.7 * batch_loss + 0.3 * mem_loss)


def _host_prep(y_true, y_pred, lookup, mem_embeddings, mem_timestamps,
               mem_initialized):
    bf16 = ml_dtypes.bfloat16
    l = np.asarray(y_true).astype(np.int64)
    yp = np.ascontiguousarray(y_pred[:, :DD]).astype(np.float32)

    emb, init, ts = _bank_update(l, yp, mem_embeddings, mem_timestamps,
                                 mem_initialized)
    m = np.where(init[:, None], _l2norm(emb), 0.0).astype(np.float32)
    w = (np.exp(-DECAY * (CUR_TIME - ts)) * init).astype(np.float32)
    n_init = max(int(init.sum()), 1)

    x = _l2norm(yp)
    xs = (x * SCALE).astype(bf16)             # [B, DD]

    # ---- initialized-classes-first permutation, padded to 512 ----
    perm = np.argsort(~init, kind="stable")   # init classes first
    CM = max(512, int(-(-n_init // 512)) * 512)
    permk = perm[:CM]
    ms = (m[permk] * SCALE).astype(bf16)      # [CM, DD]
    wk = w[permk]                             # [CM]

    # m^T in [128, KC*CM] SBUF layout (per-partition contiguous lines)
    msT3 = np.ascontiguousarray(ms.T).reshape(KC, 128, CM)
    mr = np.ascontiguousarray(msT3.transpose(1, 0, 2).reshape(128, KC * CM))

    xsT = np.ascontiguousarray(xs.T)          # [DD, B]

    in_maps = []
    R = np.empty(B, dtype=np.float64)
    for k in range(N_CORES):
        rows = slice(k * RPC, (k + 1) * RPC)
        # rotated x columns: global col (k*512 + c) mod B -> local col c
        rot = np.concatenate([xsT[:, k * 512:], xsT[:, :k * 512]],
                             axis=1)[:, :BCOLS]
        xr3 = np.ascontiguousarray(rot).reshape(KC, 128, BCOLS)
        xr = np.ascontiguousarray(
            xr3.transpose(1, 0, 2).reshape(128, KC * BCOLS))

        t_rows = lookup[l[rows]][:, permk]    # [RPC, CM] f32
        u32 = wk[None, :] * (1.0 - t_rows)    # [RPC, CM]
        R[rows] = u32.sum(axis=1, dtype=np.float64)
        u16 = u32.astype(np.float16)
        ur = np.ascontiguousarray(
            u16.reshape(RB, 128, CM).transpose(1, 0, 2).reshape(128, RB * CM))
        in_maps.append({"xr": xr, "mr": mr, "ur": ur})

    # ---- analytic pieces (f64, from the same quantized operands) ----
    xs64 = xs.astype(np.float64)
    cos_ii = (xs64 * xs64).sum(axis=1)                      # [B]
    d_ii = np.sqrt(np.maximum(2.0 - 2.0 * cos_ii, 0.0))
    Sd_self_group = d_ii.reshape(G, RPC).sum(axis=1)        # [G]
    s_vec = xs64.sum(axis=0)
    T2_upper = (B * (B - 1) // 2) * 2.0 - (s_vec @ s_vec - cos_ii.sum())

    w64 = w.astype(np.float64)
    W = w64.sum()
    s_m = (w64[:, None] * m.astype(np.float64)).sum(axis=0)
    xdots = x.astype(np.float64) @ s_m

    meta = dict(Sd_self_group=Sd_self_group, T2_upper=T2_upper, W=W,
                xdots=xdots, R=R, n_init=n_init, n_valid=B, CM=CM)
    return in_maps, meta


def _assemble(results, meta):
    S_diag = S_pure = S_d4 = 0.0
    q = np.zeros(B, dtype=np.float64)
    for k, res in enumerate(results):
        s_acc = np.asarray(res["s_acc"], dtype=np.float64)   # [128, 20]
        q_acc = np.asarray(res["q_acc"], dtype=np.float64)   # [128, RB*8]
        S_d4 += s_acc[:, 16:20].sum()
        for rb in range(RB):
            S_diag += s_acc[:, rb * 4 + 0].sum()
            S_pure += s_acc[:, rb * 4 + 1:rb * 4 + 4].sum()
            rows = slice(k * RPC + rb * 128, k * RPC + (rb + 1) * 128)
            q[rows] = q_acc[:, rb * 8:(rb + 1) * 8].sum(axis=1)

    n_pairs = B * (B - 1) // 2
    Sd_upper = (S_pure
                + 0.5 * S_d4
                + 0.5 * (S_diag - meta["Sd_self_group"].sum()))
    batch_sum = 16.0 * n_pairs - 8.0 * Sd_upper + meta["T2_upper"]
    batch_loss = batch_sum / n_pairs

    per_i = (2.0 * meta["W"] - 2.0 * meta["xdots"]) + 16.0 * meta["R"] - 8.0 * q
    mem_loss = per_i.sum() / meta["n_init"] / meta["n_valid"]
    return np.float32(0.7 * batch_loss + 0.3 * mem_loss)


# ---------------------------------------------------------------- device
def _build_nc(cm=3584, reps=1):
    key = (cm, reps)
    if key in _NC_CACHE:
        return _NC_CACHE[key]
    import concourse.bacc as bacc
    import concourse.mybir as mybir
    import concourse.tile as tile
    from concourse._compat import get_trn_type

    f32 = mybir.dt.float32
    bf16 = mybir.dt.bfloat16
    f16 = mybir.dt.float16
    Sqrt = mybir.ActivationFunctionType.Sqrt
    NCH = cm // 512                       # mem col chunks (<= 8)

    nc = bacc.Bacc(get_trn_type() or "TRN2", target_bir_lowering=False,
                   debug=False)

    xr_d = nc.dram_tensor("xr", [128, KC * BCOLS], bf16, kind="ExternalInput")
    mr_d = nc.dram_tensor("mr", [128, KC * cm], bf16, kind="ExternalInput")
    ur_d = nc.dram_tensor("ur", [128, RB * cm], f16, kind="ExternalInput")
    s_out = nc.dram_tensor("s_acc", [128, 20], f32, kind="ExternalOutput")
    q_out = nc.dram_tensor("q_acc", [128, RB * 8], f32, kind="ExternalOutput")

    # mem chunk groups per row block: [0:2048) and [2048:cm)
    if NCH > 4:
        mem_groups = [(0, 4), (4, NCH)]
    else:
        mem_groups = [(0, NCH)]

    MMW = 512                           # moving-operand width (PSUM bank)

    def mm_group(ps, wfn, colfn, j0, j1):
        """Accumulate K=384 into psum slices covering col tiles [j0, j1)."""
        width = (j1 - j0) * 512
        for kc in range(KC):
            o = 0
            while o < width:
                n = min(MMW, width - o)
                nc.tensor.matmul(
                    ps[:, o:o + n], wfn(kc), colfn(kc, o, n),
                    start=(kc == 0), stop=(kc == KC - 1))
                o += n

    with tile.TileContext(nc) as tc:
        with (
            tc.tile_pool(name="const", bufs=1) as const,
            tc.tile_pool(name="inp", bufs=min(reps, 2)) as inp,
            tc.tile_pool(name="psum", bufs=2, space="PSUM") as psum,
            tc.tile_pool(name="work", bufs=3) as work,
        ):
            for rep in range(reps):
                # Split input DMAs so compute can start after the first
                # slice lands; issue in consumption order.
                xr = inp.tile([128, KC * BCOLS], bf16, tag="xr")
                for kc in range(KC):
                    o = kc * BCOLS
                    # first piece covers weights + first matmul columns so
                    # the PE can start while the rest streams in; the d4
                    # columns [2048:2560) are only needed at the very end
                    nc.sync.dma_start(xr[:, o:o + 1024], xr_d[:, o:o + 1024])
                    nc.sync.dma_start(xr[:, o + 1024:o + 2048],
                                      xr_d[:, o + 1024:o + 2048])
                mr = inp.tile([128, KC * cm], bf16, tag="mr")
                for kc in range(KC):
                    nc.sync.dma_start(mr[:, kc * cm:(kc + 1) * cm],
                                      mr_d[:, kc * cm:(kc + 1) * cm])
                ur = inp.tile([128, RB * cm], f16, tag="ur")
                for rb in range(RB):
                    nc.sync.dma_start(ur[:, rb * cm:(rb + 1) * cm],
                                      ur_d[:, rb * cm:(rb + 1) * cm])
                for kc in range(KC):
                    o = kc * BCOLS
                    nc.sync.dma_start(xr[:, o + 2048:o + BCOLS],
                                      xr_d[:, o + 2048:o + BCOLS])

                s_acc = const.tile([128, 20], f32, tag="s_acc")
                q_acc = const.tile([128, RB * 8], f32, tag="q_acc")
                nc.vector.memset(q_acc[:], 0.0)
                bias2 = const.tile([128, 1], f32, tag="bias2")
                nc.vector.memset(bias2[:], 2.0)

                def xw(kc, rb):
                    # weights: own rows = rotated cols [rb*128, rb*128+128)
                    o = kc * BCOLS + rb * 128
                    return xr[:, o:o + 128]

                def emit_batch(rb):
                    # rotated col tiles 0..3: own diagonal block + 3 pure
                    ps = psum.tile([128, 2048], f32, tag="ps")
                    mm_group(ps, lambda kc: xw(kc, rb),
                             lambda kc, o, n: xr[:, kc * BCOLS + o:
                                                 kc * BCOLS + o + n], 0, 4)
                    db = work.tile([128, 2048], bf16, tag="db")
                    for j in range(4):
                        o = j * 512
                        nc.scalar.activation(
                            db[:, o:o + 512], ps[:, o:o + 512], Sqrt,
                            bias=bias2[:], scale=-2.0,
                            accum_out=s_acc[:, rb * 4 + j:rb * 4 + j + 1])

                def emit_mem(rb, gi):
                    j0, j1 = mem_groups[gi]
                    width = (j1 - j0) * 512
                    ps = psum.tile([128, 2048], f32, tag="ps")
                    mm_group(ps, lambda kc: xw(kc, rb),
                             lambda kc, o, n: mr[:, kc * cm + j0 * 512 + o:
                                                 kc * cm + j0 * 512 + o + n],
                             j0, j1)
                    dm = work.tile([128, 2048], bf16, tag="dm")
                    for j in range(j0, j1):
                        o = (j - j0) * 512
                        nc.scalar.activation(
                            dm[:, o:o + 512], ps[:, o:o + 512], Sqrt,
                            bias=bias2[:], scale=-2.0)
                    junk = work.tile([128, 2048], bf16, tag="junk")
                    uo = rb * cm + j0 * 512
                    nc.vector.tensor_tensor(
                        junk[:, 0:width], dm[:, 0:width],
                        ur[:, uo:uo + width], mybir.AluOpType.mult)
                    nc.vector.tensor_reduce(
                        q_acc[:, rb * 8 + gi:rb * 8 + gi + 1],
                        junk[:, 0:width], mybir.AxisListType.XYZW,
                        mybir.AluOpType.add)

                def emit_d4():
                    # rotated col tile 4 of all four row blocks in one big
                    # tile (antipodal blocks, double-counted; host halves).
                    ps4 = psum.tile([128, 2048], f32, tag="ps")
                    for rb in range(RB):
                        for kc in range(KC):
                            o = rb * 512
                            nc.tensor.matmul(
                                ps4[:, o:o + 512], xw(kc, rb),
                                xr[:, kc * BCOLS + 2048:kc * BCOLS + 2560],
                                start=(kc == 0), stop=(kc == KC - 1))
                    db4 = work.tile([128, 2048], bf16, tag="db")
                    for rb4 in range(RB):
                        o = rb4 * 512
                        nc.scalar.activation(
                            db4[:, o:o + 512], ps4[:, o:o + 512], Sqrt,
                            bias=bias2[:], scale=-2.0,
                            accum_out=s_acc[:, 16 + rb4:17 + rb4])

                # Batch tiles first (they only need xr, which lands first),
                # then mem tiles; d4 is the cheap tail (single ACT, no TTR).
                sched = [("b", rb) for rb in range(RB)]
                for rb in range(RB):
                    for gi in range(len(mem_groups)):
                        sched.append(("m", rb, gi))
                sched.append(("d4",))
                for item in sched:
                    if item[0] == "b":
                        emit_batch(item[1])
                    elif item[0] == "m":
                        emit_mem(item[1], item[2])
                    else:
                        emit_d4()

                nc.sync.dma_start(s_out[:], s_acc[:])
                nc.sync.dma_start(q_out[:], q_acc[:])

    nc.compile()
    _NC_CACHE[key] = nc
    return nc


def kernel(y_true, y_pred, lookup, mem_embeddings, mem_timestamps,
           mem_initialized):
    y_true = np.asarray(y_true)
    y_pred = np.asarray(y_pred, dtype=np.float32)
    lookup = np.asarray(lookup, dtype=np.float32)
    mem_embeddings = np.asarray(mem_embeddings, dtype=np.float32)
    mem_timestamps = np.asarray(mem_timestamps, dtype=np.float32)
    mem_initialized = np.asarray(mem_initialized, dtype=np.int32)

    l = y_true.astype(np.int64)
    if (y_pred.shape != (B, D) or lookup.shape != (C, C)
            or mem_embeddings.shape != (C, DD)
            or not ((l >= 0) & (l < C)).all()):
        return _numpy_fallback(y_true, y_pred, lookup, mem_embeddings,
                               mem_timestamps, mem_initialized)

    from concourse.bass_utils import run_bass_kernel_spmd

    in_maps, meta = _host_prep(y_true, y_pred, lookup, mem_embeddings,
                               mem_timestamps, mem_initialized)
    nc = _build_nc(cm=meta["CM"])
    res = run_bass_kernel_spmd(nc, in_maps, list(range(N_CORES)),
                               trace=TRACE)
    LAST_RESULTS["bass"] = res
    return _assemble(res.results, meta)
